# revision 1
# baseline (speedup 1.0000x reference)
"""BEV voxel-pooling (segment_reduce) kernel for 8 Trainium2 NeuronCores. v3

Host (numpy, layout only — no feature arithmetic):
  * compute per-point BEV rank exactly as the reference does
  * per sample, stable-sort points by rank; 4 shards per sample snapped to
    rank boundaries (8 cores)
  * per core, FFD bin-pack segments (one segment = one rank) into blocks of
    <= 1024 points and <= 128 segments; blocks sorted by chunk count so the
    cross-core schedule profile (chunks U_i, one-hot width s per pair) is
    tight; all cores share one static schedule
  * features uploaded fp16 in schedule order; lseg duplicated in pairs so
    the batched one-hot build qualifies for the DVE 2x perf mode

Device (per core, one SPMD Bass/Tile program, fully static):
  * feature table resident in SBUF, streamed in pair-aligned DMA pieces
    (small first pieces so compute starts early); all on the hardware DGE
  * one batched DVE tensor_tensor(is_equal) builds a block-PAIR's one-hots;
    it also carries the dep on its feature piece so matmuls need only one
    cross-engine wait
  * per block, U_b fp16 matmuls accumulate into the block's 64-col slice of
    a per-QUAD PSUM tile (distinct tiles -> no false inter-quad deps);
    K=1 zero-matmuls pre-zero each quad and double as the PE clock ramp
  * per quad: ACT copy PSUM->SBUF (own stage tile), then one plain
    dma_start (SP queue, hardware DGE) to the window-major output
Host gather: place each block's rows at their ranks in the (B,40000,64)
grid (pure indexing), reshape to (B, C, X, Y).
"""
import sys
sys.path.insert(0, '/opt/trn_rl_repo')

import numpy as np

# ---------------- problem constants (hardcoded per spec) ----------------
B, N, C = 2, 6, 64
H_IMG, W_IMG = 256, 704
DS = 16
DSH, DSW = H_IMG // DS, W_IMG // DS          # 16, 44
D0, D1 = 4, 45                                # depth bins -> D = 41
X, Y, Z = 200, 200, 1
NBINS = X * Y * Z
NP_SAMPLE = N * (D1 - D0) * DSH * DSW         # 173184
NCORES = 8
SHARDS_PER_SAMPLE = 4

CAP = 1024             # point capacity per block
SEG_LIMIT = 128        # max segments per block (PSUM partition limit)

_compiled = {}


# ---------------- host geometry (matches reference numerics) ----------------
def _compute_ranks(frustum, post_trans, post_rots, intrinsics, extrinsics,
                   bev_res, bev_start_pos):
    frustum = np.asarray(frustum, np.float32)
    post_trans = np.asarray(post_trans, np.float32)
    post_rots = np.asarray(post_rots, np.float32)
    intrinsics = np.asarray(intrinsics, np.float32)
    extrinsics = np.asarray(extrinsics, np.float32)
    bev_res = np.asarray(bev_res, np.float32)
    bev_start_pos = np.asarray(bev_start_pos, np.float32)

    ext_inv = np.linalg.inv(extrinsics.astype(np.float64)).astype(np.float32)
    rot = ext_inv[..., :3, :3]
    trans = ext_inv[..., :3, 3]
    pts = frustum[None, None] - post_trans[:, :, None, None, None, :]
    pr_inv = np.linalg.inv(post_rots.astype(np.float64)).astype(np.float32)
    pts = np.einsum('bnij,bndhwj->bndhwi', pr_inv, pts).astype(np.float32)
    pts = np.concatenate([pts[..., :2] * pts[..., 2:3], pts[..., 2:3]], axis=-1)
    comb = (rot @ np.linalg.inv(intrinsics.astype(np.float64)).astype(np.float32)
            ).astype(np.float32)
    pts = np.einsum('bnij,bndhwj->bndhwi', comb, pts).astype(np.float32)
    geom = pts + trans[:, :, None, None, None, :]

    coords = (geom - (bev_start_pos - bev_res / 2.0)) / bev_res
    ci = coords.reshape(B, -1, 3).astype(np.int32)
    mask = ((ci[..., 0] >= 0) & (ci[..., 0] < X) &
            (ci[..., 1] >= 0) & (ci[..., 1] < Y) &
            (ci[..., 2] >= 0) & (ci[..., 2] < Z))
    rank = ci[..., 0] * (Y * Z) + ci[..., 1] * Z + ci[..., 2]
    return rank, mask


# ---------------- host planning ----------------
class CorePlan:
    __slots__ = ("sample", "blocks")
    # blocks: list of (ranks, point_indices, local_seg)


def _plan_cores(rank, mask):
    plans = []
    for b in range(B):
        r = rank[b]
        m = mask[b]
        valid_idx = np.nonzero(m)[0]
        order = valid_idx[np.argsort(r[valid_idx], kind='stable')]
        rs = r[order]
        P = len(order)
        cuts = [0]
        for s in range(1, SHARDS_PER_SAMPLE):
            i = s * P // SHARDS_PER_SAMPLE
            while i < P and rs[i] == rs[i - 1]:
                i += 1
            cuts.append(i)
        cuts.append(P)
        for s in range(SHARDS_PER_SAMPLE):
            pl = CorePlan()
            pl.sample = b
            lo, hi = cuts[s], cuts[s + 1]
            sl_order = order[lo:hi]
            sl_rs = rs[lo:hi]
            if len(sl_rs):
                newseg = np.r_[True, sl_rs[1:] != sl_rs[:-1]]
                seg_starts = np.nonzero(newseg)[0]
                seg_counts = np.diff(np.r_[seg_starts, len(sl_rs)])
                seg_ranks = sl_rs[seg_starts]
            else:
                seg_starts = seg_counts = seg_ranks = np.zeros(0, np.int64)
            desc = np.argsort(-seg_counts, kind='stable')
            bins = []
            for si in desc:
                c = int(seg_counts[si])
                placed = False
                for bn in bins:
                    if bn[0] + c <= CAP and len(bn[1]) < SEG_LIMIT:
                        bn[0] += c
                        bn[1].append(si)
                        placed = True
                        break
                if not placed:
                    bins.append([c, [si]])
            # sort by chunk count desc (primary) then seg count desc
            bins.sort(key=lambda bn: (-((bn[0] + 127) // 128), -len(bn[1])))
            blocks = []
            for bn in bins:
                segs = bn[1]
                ranks = seg_ranks[segs]
                pts = np.concatenate(
                    [np.arange(seg_starts[si], seg_starts[si] + seg_counts[si])
                     for si in segs])
                lseg = np.concatenate(
                    [np.full(int(seg_counts[si]), j, np.int64)
                     for j, si in enumerate(segs)])
                blocks.append((ranks, sl_order[pts], lseg))
            pl.blocks = blocks
            plans.append(pl)

    NB = max(len(pl.blocks) for pl in plans)
    NB += -NB % 8          # octet PSUM banks need NB % 8 == 0
    for pl in plans:       # pad to a full uniform schedule
        z = np.zeros(0, np.int64)
        while len(pl.blocks) < NB:
            pl.blocks.append((z, z, z))
    U_prof = np.ones(NB, np.int64)
    S_blk = np.zeros(NB, np.int64)
    for pl in plans:
        for i, (ranks, pts, _) in enumerate(pl.blocks):
            U_prof[i] = max(U_prof[i], (len(pts) + 127) // 128)
            S_blk[i] = max(S_blk[i], len(ranks))
    S_pair = []
    for i in range(NB // 2):
        s = max(int(S_blk[2 * i]), int(S_blk[2 * i + 1]), 2)
        S_pair.append(s + (s % 2))
    # rotate the octet with the cheapest first one-hot build to the front
    # so the first matmuls start as early as possible
    nq = NB // 8
    rest = list(range(2, nq))
    last = min(rest, key=lambda j: max(S_pair[4 * j:4 * j + 4]))
    perm = [0, 1] + [j for j in rest if j != last] + [last]
    U_prof = np.concatenate([U_prof[8 * j:8 * j + 8] for j in perm])
    S_pair = sum([S_pair[4 * j:4 * j + 4] for j in perm], [])
    for pl in plans:
        pl.blocks = sum([pl.blocks[8 * j:8 * j + 8] for j in perm], [])
    return plans, NB, tuple(int(u) for u in U_prof), tuple(S_pair)


def _schedule(NB, U_prof):
    """Chunk offsets, per-chunk byte offsets and piece boundaries (bytes).

    The first two octets (blocks 0..15, the biggest-U ones) ship as e4m3:
    an fp8 PREFIX rides the PE's cold-start clock ramp, so the single
    fp8->fp16 dtype switch is the only pipeline flush in the stream."""
    coff = np.r_[0, np.cumsum(U_prof)]
    NCH = int(coff[-1])
    fp8_lim = int(coff[16])        # chunks of blocks 0..15 are e4m3
    B0 = 256 + 4 * NCH             # iota (256B) + lseg2 (4B/chunk-pair col)
    fboff = np.zeros(NCH + 1, np.int64)
    fboff[0] = B0
    for c in range(NCH):
        fboff[c + 1] = fboff[c] + (C if c < fp8_lim else 2 * C)
    NBP = NB // 2
    # pieces aligned to OCTET boundaries (writeback fires as pieces land);
    # the FINAL octet is split 2+2 so the PE trail after the last DMA
    # semaphore is half an octet, not a whole one
    pieces = [(0, 1), (1, 4)]
    for p in range(4, NBP - 4, 4):
        pieces.append((p, p + 4))
    pieces.append((NBP - 4, NBP - 2))
    pieces.append((NBP - 2, NBP))
    return coff, NCH, B0, fboff, pieces, fp8_lim


def _build_inputs(pl, feats_b, NB, U_prof, S_pair):
    import ml_dtypes
    coff, NCH, B0, fboff, _, fp8_lim = _schedule(NB, U_prof)
    blob = np.zeros((128, int(fboff[-1])), np.uint8)
    iota = np.broadcast_to(np.arange(128, dtype=np.float16), (128, 128))
    blob[:, 0:256] = np.ascontiguousarray(iota).view(np.uint8)
    lseg2 = np.full((128, NCH * 2), 255.0, np.float16)
    for i, (ranks, pts, lseg) in enumerate(pl.blocks):
        n = len(pts)
        if not n:
            continue
        f = feats_b[pts].astype(np.float16)
        nch = (n + 127) // 128
        fpad = np.zeros((nch * 128, C), np.float16)
        fpad[:n] = f
        lpad = np.full(nch * 128, 255, np.int64)
        lpad[:n] = lseg
        c0 = int(coff[i])
        for k in range(nch):
            c = c0 + k
            ch = fpad[k * 128:(k + 1) * 128]           # [128, C] fp16
            if c < fp8_lim:
                b = ch.astype(ml_dtypes.float8_e4m3fn).view(np.uint8)
            else:
                b = ch.view(np.uint8)
            blob[:, int(fboff[c]):int(fboff[c + 1])] = b
        lv = lpad.reshape(nch, 128).T.astype(np.float16)
        lseg2[:, 2 * c0:2 * (c0 + nch)] = np.repeat(lv, 2, axis=1)
    blob[:, 256:B0] = lseg2.view(np.uint8)
    return {"blob": blob.view(ml_dtypes.float8_e4m3fn)}


# ---------------- device program ----------------
def _build_kernel(NB, U_prof, S_pair):
    import concourse.bass as bass
    import concourse.bacc as bacc
    import concourse.mybir as mybir
    import concourse.tile as tile
    from concourse.tile_rust import add_dep_helper
    from contextlib import ExitStack

    F32 = mybir.dt.float32
    F16 = mybir.dt.float16
    coff, NCH, B0, fboff, pieces, fp8_lim = _schedule(NB, U_prof)
    NBP = NB // 2
    S_OCT = [max(S_pair[4 * j:4 * j + 4]) for j in range(NB // 8)]
    oh_cols = [(int(coff[2 * i + 2] - coff[2 * i])) * S_OCT[i // 4]
               for i in range(NBP)]
    oh_off = np.r_[0, np.cumsum(oh_cols)]
    OH_TOT = int(oh_off[-1])
    F8 = mybir.dt.float8e4
    TOT = int(fboff[-1])

    nc = bacc.Bacc()
    blob = nc.dram_tensor("blob", [128, TOT], F8, kind="ExternalInput")
    out = nc.dram_tensor("out", [128, NB * 8 * C], F16, kind="ExternalOutput")

    with tile.TileContext(nc) as tc, ExitStack() as ctx:
        const = ctx.enter_context(tc.tile_pool(name="const", bufs=1))

        blob_sb = const.tile([128, TOT], F8)
        iota_sb = blob_sb[:, 0:256].bitcast(F16)
        lseg2_sb = blob_sb[:, 256:B0].bitcast(F16)

        # pair-aligned DMA pieces; piece 0 carries iota+lseg2+first pair
        feat_dmas = []
        piece_of_pair = {}
        for pz, (pa, pb) in enumerate(pieces):
            a = 0 if pz == 0 else int(fboff[int(coff[2 * pa])])
            b_ = int(fboff[int(coff[2 * pb])])
            eng = nc.sync
            feat_dmas.append(eng.dma_start(blob_sb[:, a:b_],
                                           blob[:, a:b_]))
            for i in range(pa, pb):
                piece_of_pair[i] = pz

        oh_all = const.tile([128, OH_TOT], F16)
        stages = [const.tile([128, 8 * C], F16, name=f"stage{j}")
                  for j in range(NB // 8)]

        psump = ctx.enter_context(
            tc.tile_pool(name="psum", bufs=1, space="PSUM"))
        quads = [psump.tile([128, 8 * C], F32, name=f"quad{j}", tag=f"q{j}")
                 for j in range(NB // 8)]

        for i in range(NBP):
            s = S_OCT[i // 4]
            off = int(oh_off[i])
            cnt = int(coff[2 * i + 2] - coff[2 * i])
            c0 = int(coff[2 * i])
            ov = oh_all[:, off:off + cnt * s].rearrange(
                "p (u j r) -> p u j r", u=cnt, r=2)
            i0 = (iota_sb[:, 0:s].rearrange("p (j r) -> p j r", r=2)
                  .unsqueeze(1).broadcast_to([128, cnt, s // 2, 2]))
            l1 = (lseg2_sb[:, 2 * c0:2 * (c0 + cnt)]
                  .rearrange("p (u r) -> p u r", r=2)
                  .unsqueeze(2).broadcast_to([128, cnt, s // 2, 2]))
            nc.vector.tensor_tensor(ov, i0, l1, mybir.AluOpType.is_equal)

            qt = quads[i // 4]
            for half in range(2):
                b_ = 2 * i + half
                ub = int(U_prof[b_])
                col = (b_ % 8) * C
                for u in range(ub):
                    c = int(coff[b_]) + u
                    fsl = blob_sb[:, int(fboff[c]):int(fboff[c + 1])]
                    rhs = fsl if c < fp8_lim else fsl.bitcast(F16)
                    nc.tensor.matmul(
                        qt[0:s, col:col + C],
                        oh_all[:, off + (c - c0) * s:off + (c - c0 + 1) * s],
                        rhs,
                        start=(u == 0), stop=True, skip_group_check=True)

            if i % 4 == 3:
                j = i // 4
                so = S_OCT[j]
                nc.scalar.copy(stages[j][0:so, :], quads[j][0:so, :])
                eng = nc.scalar
                eng.dma_start(
                    out[0:so, j * 8 * C:(j + 1) * 8 * C],
                    stages[j][0:so, :])
    nc.finalize()
    return nc


# ---------------- entry point ----------------
def kernel(image_feature, post_trans, post_rots, intrinsics, extrinsics,
           frustum, bev_res, bev_start_pos):
    from concourse.bass_utils import run_bass_kernel_spmd
    import os

    rank, mask = _compute_ranks(frustum, post_trans, post_rots, intrinsics,
                                extrinsics, bev_res, bev_start_pos)
    feats = np.ascontiguousarray(np.asarray(image_feature, np.float32)
                                 .reshape(B, NP_SAMPLE, C))
    plans, NB, U_prof, S_pair = _plan_cores(rank, mask)

    in_maps = [_build_inputs(pl, feats[pl.sample], NB, U_prof, S_pair)
               for pl in plans]

    key = (NB, U_prof, S_pair)
    if key not in _compiled:
        _compiled[key] = _build_kernel(*key)
    nc = _compiled[key]

    trace = bool(int(os.environ.get("BEV_TRACE", "0")))
    res = run_bass_kernel_spmd(nc, in_maps, core_ids=list(range(NCORES)),
                               trace=trace,
                               trace_cores=[0] if trace else None)
    if trace and res.exec_time_ns is not None:
        print(f"HW exec time: {res.exec_time_ns} ns")
        kernel.last_exec_time_ns = res.exec_time_ns
        kernel.last_results = res

    grid = np.zeros((B, NBINS, C), np.float32)
    for k, pl in enumerate(plans):
        o = res.results[k]["out"]
        for i, (ranks, _, _) in enumerate(pl.blocks):
            n = len(ranks)
            if n:
                grid[pl.sample, ranks] = o[:n, i * C:(i + 1) * C]
    return np.ascontiguousarray(
        grid.reshape(B, X, Y, C).transpose(0, 3, 1, 2))

